# revision 6
# baseline (speedup 1.0000x reference)
"""Causal self-attention with RoPE on 8 Trainium2 NeuronCores.

Sharding: tensor-parallel over heads x data-parallel over batch.
  core c -> batch b = c // 2, head-group g = c % 2 (heads 8g .. 8g+7).
Each core computes qkv projections for its 8 heads, RoPE, causal
attention, and a *partial* output projection (its heads' contribution
to y[b]). Host sums the two partials per batch and adds the bias
terms (b_proj and the v-bias routed through W_proj).

v2 design (vs the f32r baseline):
  * All PE operands are bf16 (1 cycle/row, same as f32r, but half the
    DMA/SBUF footprint). PSUM accumulation stays fp32.
  * Host prepacks x/weights into the exact SBUF tile layouts so every
    DMA moves large contiguous per-partition segments.
  * Phase A is a single fused pass over x^T chunks: q, k and v
    projections share each x chunk (x read once, not three times).
    RoPE rotate-matmuls are software-pipelined one head behind the
    accumulation groups so the PE never waits on the ACT eviction.
  * Attention runs q-chunk-outer / head-inner.  Softmax denominators
    are built by accumulating the exp tiles on the Vector engine and
    reducing with ONE ones-matmul per (head, q-chunk) instead of one
    per k-block (saves ~70us of PE time).  Causal masking multiplies
    bf16 0/1 masks on the DVE fast path.
  * The output projection for q-chunk qc-1 is interleaved into the
    attention head loop of q-chunk qc, so the proj never waits and
    PE gaps from the exp dependency chain get filled.
"""

import numpy as np
import ml_dtypes

import concourse.bass as bass
import concourse.mybir as mybir
import concourse.tile as tile
from concourse import bacc
from concourse.bass_utils import run_bass_kernel_spmd

F32 = mybir.dt.float32
F32R = mybir.dt.float32r
BF16 = mybir.dt.bfloat16
AF = mybir.ActivationFunctionType
ALU = mybir.AluOpType

D_MODEL = 2048
N_HEADS = 16
HD = 128
B, T = 4, 2048
N_CORES = 8
HPC = 8           # heads per core
TQC = 512         # query-chunk (free dim of S^T blocks)
PB = 128          # partitions / k-chunk
SCALE = 1.0 / np.sqrt(HD)
NPBF = np.dtype(ml_dtypes.bfloat16)


def build_nc(t=T, d=D_MODEL, hpc=HPC, tqc=TQC, compile=True):
    """Build the per-core Bass module. All 8 cores run this same module on
    different input slices."""
    nc = bacc.Bacc(trn_type="TRN2", target_bir_lowering=False)

    dck = d // PB          # D-chunks (contraction tiles)
    ntc = t // tqc         # t-chunks of tqc
    ntc128 = t // PB       # t-chunks of 128
    nmask = tqc // PB      # partial-block masks per q-chunk
    nvc = hpc * HD // tqc  # v output column chunks (2)
    hps = tqc // HD        # heads per v column chunk (4)
    npq = d // tqc         # proj output column chunks of tqc

    # Host-prepacked inputs (bf16, partition-major, contiguous segments).
    xp = nc.dram_tensor("xp", [PB, ntc, dck, tqc], BF16, kind="ExternalInput")
    wq = nc.dram_tensor("wq", [PB, hpc, dck, HD], BF16, kind="ExternalInput")
    wk = nc.dram_tensor("wk", [PB, hpc, dck, HD], BF16, kind="ExternalInput")
    wv = nc.dram_tensor("wv", [PB, nvc, dck, tqc], BF16, kind="ExternalInput")
    wp = nc.dram_tensor("wp", [PB, npq, hpc, tqc], BF16, kind="ExternalInput")
    bq = nc.dram_tensor("bq", [HD, hpc], F32, kind="ExternalInput")
    bk = nc.dram_tensor("bk", [HD, hpc], F32, kind="ExternalInput")
    cosT = nc.dram_tensor("cosT", [HD, t], BF16, kind="ExternalInput")
    # sinTs is sign-folded on host: rows 0:64 negated.
    sinTs = nc.dram_tensor("sinTs", [HD, t], BF16, kind="ExternalInput")
    y = nc.dram_tensor("y", [t, d], F32, kind="ExternalOutput")

    with tile.TileContext(nc) as tc:
        with (
            tc.tile_pool(name="dram", bufs=1, space="DRAM") as dram,
            tc.tile_pool(name="consts", bufs=1) as consts,
            tc.tile_pool(name="vall", bufs=1) as pva,
        ):
            qT_d = dram.tile([hpc, HD, t], BF16, tag="qT_d")
            kT_d = dram.tile([hpc, HD, t], BF16, tag="kT_d")
            # v stays resident in SBUF from phase A through attention.
            v_all = pva.tile([PB, ntc128, hpc, HD], BF16, tag="v_all")

            # --- constants -------------------------------------------------
            ones_f = consts.tile([PB, 1], F32, tag="ones_f")
            nc.vector.memset(ones_f, 1.0)
            ones_col = consts.tile([PB, 1], F32R, tag="ones")
            nc.vector.tensor_copy(ones_col, ones_f)
            # rotate_half as a matmul constant: rot^T = RT.T @ qT with
            # RT a pure 64-rotation permutation (signs live in sinTs,
            # whose first 64 rows are negated on the host).
            rT_f = consts.tile([HD, HD], F32, tag="rT_f")
            nc.gpsimd.memset(rT_f, 0.0)
            nc.gpsimd.affine_select(
                out=rT_f, in_=rT_f, compare_op=ALU.not_equal, fill=1.0,
                base=64, pattern=[[1, HD]], channel_multiplier=-1,
            )
            nc.gpsimd.affine_select(
                out=rT_f, in_=rT_f, compare_op=ALU.not_equal, fill=1.0,
                base=-64, pattern=[[1, HD]], channel_multiplier=-1,
            )
            rT = consts.tile([HD, HD], BF16, tag="rT")
            nc.vector.tensor_copy(rT, rT_f)
            # causal masks for the nmask partial diagonal blocks:
            # mask_r[p, f] = 1.0 if f >= p + r*128 else 0.0  (bf16)
            masks = []
            mk_f = consts.tile([PB, tqc], F32, tag="mk_f")
            for r in range(nmask):
                nc.gpsimd.memset(mk_f, 1.0)
                nc.gpsimd.affine_select(
                    out=mk_f, in_=mk_f,
                    compare_op=ALU.is_ge,
                    fill=0.0,
                    base=-(r * PB),
                    pattern=[[1, tqc]],
                    channel_multiplier=-1,
                )
                mk = consts.tile([PB, tqc], BF16, tag=f"mask{r}", name=f"mask{r}")
                nc.vector.tensor_copy(mk, mk_f)
                masks.append(mk)

            cosT_s = consts.tile([HD, t], BF16, tag="cosT")
            sinT_s = consts.tile([HD, t], BF16, tag="sinT")
            bq_s = consts.tile([HD, hpc], F32, tag="bq")
            bk_s = consts.tile([HD, hpc], F32, tag="bk")

            # ================= Phase A: fused QKV projections =============
            with (
                tc.tile_pool(name="w_q", bufs=1) as pwq,
                tc.tile_pool(name="w_k", bufs=1) as pwk,
                tc.tile_pool(name="w_v", bufs=1) as pwv,
                tc.tile_pool(name="x_a", bufs=2) as px,
                tc.tile_pool(name="raw_a", bufs=3) as pra,
                tc.tile_pool(name="rope_a", bufs=3) as pro,
                tc.tile_pool(name="psA", bufs=1, space="PSUM") as psA,
            ):
                wq_s = pwq.tile([PB, hpc, dck, HD], BF16, tag="wq_s")
                wk_s = pwk.tile([PB, hpc, dck, HD], BF16, tag="wk_s")
                wv_s = pwv.tile([PB, nvc, dck, tqc], BF16, tag="wv_s")

                # First-consumed-first DMA order: interleave x chunk 0 and
                # wq head 0 at fine grain so the first accumulation group
                # starts within ~2us, then remaining wq heads, wk, wv.
                xt_tiles = [None] * ntc
                xt_tiles[0] = px.tile([PB, dck, tqc], BF16, tag="xt", name="xt")
                for cc in range(0, dck, 4):
                    nc.sync.dma_start(
                        out=xt_tiles[0][:, cc:cc + 4, :],
                        in_=xp.ap()[:, 0, cc:cc + 4, :],
                    )
                    nc.sync.dma_start(
                        out=wq_s[:, 0, cc:cc + 4, :],
                        in_=wq.ap()[:, 0, cc:cc + 4, :],
                    )
                for h in range(1, hpc):
                    nc.sync.dma_start(out=wq_s[:, h], in_=wq.ap()[:, h])
                nc.sync.dma_start(out=cosT_s, in_=cosT.ap())
                nc.sync.dma_start(out=sinT_s, in_=sinTs.ap())
                nc.sync.dma_start(out=bq_s, in_=bq.ap())
                nc.sync.dma_start(out=bk_s, in_=bk.ap())
                for h in range(hpc):
                    nc.sync.dma_start(out=wk_s[:, h], in_=wk.ap()[:, h])
                for v in range(nvc):
                    nc.sync.dma_start(out=wv_s[:, v], in_=wv.ap()[:, v])

                def rope_finish(kind, h, tci, raw):
                    """rot-matmul + rope combine + DMA out for one head-chunk."""
                    ts = slice(tci * tqc, (tci + 1) * tqc)
                    ps_r = psA.tile([HD, tqc], F32, tag="ps_r", name="ps_r",
                                    bufs=2)
                    nc.tensor.matmul(ps_r, lhsT=rT, rhs=raw, start=True,
                                     stop=True)
                    rsin = pro.tile([HD, tqc], BF16, tag="rsin", name="rsin")
                    nc.vector.tensor_mul(rsin, ps_r, sinT_s[:, ts])
                    cosq = pro.tile([HD, tqc], BF16, tag="cosq", name="cosq")
                    nc.vector.tensor_mul(cosq, raw, cosT_s[:, ts])
                    out_t = pro.tile([HD, tqc], BF16, tag="outT", name="out_t")
                    nc.vector.tensor_add(out_t, cosq, rsin)
                    dst = qT_d if kind == "q" else kT_d
                    nc.sync.dma_start(out=dst[h, :, ts], in_=out_t)

                for tci in range(ntc):
                    xt_s = xt_tiles[tci]
                    for kind, w_s, bias_s in (("q", wq_s, bq_s),
                                              ("k", wk_s, bk_s)):
                        pend = None
                        for h in range(hpc):
                            ps = psA.tile([PB, tqc], F32, tag="ps_a",
                                          name="ps_a", bufs=3)
                            for c in range(dck):
                                nc.tensor.matmul(
                                    ps,
                                    lhsT=w_s[:, h, c, :],
                                    rhs=xt_s[:, c, :],
                                    start=(c == 0),
                                    stop=(c == dck - 1),
                                )
                            raw = pra.tile([PB, tqc], BF16, tag="raw",
                                           name="raw")
                            nc.scalar.activation(
                                out=raw, in_=ps, func=AF.Identity,
                                bias=bias_s[:, h:h + 1], scale=1.0,
                            )
                            if pend is not None:
                                rope_finish(kind, h - 1, tci, pend)
                            pend = raw
                        rope_finish(kind, hpc - 1, tci, pend)
                        if kind == "q":
                            # prefetch next x chunk mid-iteration
                            if tci + 1 < ntc:
                                nxt = px.tile([PB, dck, tqc], BF16, tag="xt",
                                              name="xt")
                                for cc in range(0, dck, 4):
                                    nc.sync.dma_start(
                                        out=nxt[:, cc:cc + 4, :],
                                        in_=xp.ap()[:, tci + 1, cc:cc + 4, :],
                                    )
                                xt_tiles[tci + 1] = nxt
                    # v projection for this x chunk
                    for t128 in range(tqc // PB):
                        kcg = tci * (tqc // PB) + t128
                        for nci in range(nvc):
                            ps_v = psA.tile([PB, tqc], F32, tag="ps_v",
                                            name="ps_v", bufs=3)
                            for c in range(dck):
                                nc.tensor.matmul(
                                    ps_v,
                                    lhsT=xt_s[:, c, t128 * PB:(t128 + 1) * PB],
                                    rhs=wv_s[:, nci, c, :],
                                    start=(c == 0),
                                    stop=(c == dck - 1),
                                )
                            nc.scalar.copy(
                                v_all[:, kcg, nci * hps:(nci + 1) * hps, :],
                                ps_v,
                            )

            # =========== Phase B+C: attention + interleaved proj ==========
            with (
                tc.tile_pool(name="ktq", bufs=1) as pkt,
                tc.tile_pool(name="qt_p", bufs=3) as pqt,
                tc.tile_pool(name="ot", bufs=1) as pot,
                tc.tile_pool(name="wp_p", bufs=1) as pwp,
                tc.tile_pool(name="pt_pool", bufs=4) as pp,
                tc.tile_pool(name="acc_p", bufs=2) as pac,
                tc.tile_pool(name="small", bufs=2) as psm,
                tc.tile_pool(name="yout", bufs=3) as py,
                tc.tile_pool(name="psB", bufs=1, space="PSUM") as psB,
            ):
                wp_s = pwp.tile([PB, npq, hpc, tqc], BF16, tag="wp_s")
                for nci in range(npq):
                    nc.sync.dma_start(out=wp_s[:, nci], in_=wp.ap()[:, nci])
                kt_res = [
                    pkt.tile([HD, t], BF16, tag=f"kt{h}", name=f"kt{h}")
                    for h in range(hpc)
                ]
                ot_tiles = [
                    pot.tile([HD, t], BF16, tag=f"ot{h}", name=f"ot{h}")
                    for h in range(hpc)
                ]

                def proj_group(qproj, g):
                    """One output-projection PSUM group for q-range qproj."""
                    t128 = 4 * qproj + g // npq
                    nci = g % npq
                    ps_y = psB.tile([PB, tqc], F32, tag="ps_y", name="ps_y",
                                    bufs=2)
                    for i in range(hpc):
                        hh = (7 + i) % hpc  # start with the freshest head
                        nc.tensor.matmul(
                            ps_y,
                            lhsT=ot_tiles[hh][:, t128 * PB:(t128 + 1) * PB],
                            rhs=wp_s[:, nci, hh, :],
                            start=(i == 0),
                            stop=(i == hpc - 1),
                        )
                    y_t = py.tile([PB, tqc], F32, tag="y_t", name="y_t")
                    nc.vector.tensor_copy(y_t, ps_y)
                    nc.sync.dma_start(
                        out=y.ap()[t128 * PB:(t128 + 1) * PB,
                                   nci * tqc:(nci + 1) * tqc],
                        in_=y_t,
                    )

                def finish_head(qc, h, acc, ps_o):
                    """Z-matmul + normalize for a finished head (emitted one
                    head late so the PE never waits on the exp/acc chain)."""
                    qs = slice(qc * tqc, (qc + 1) * tqc)
                    ps_z = psB.tile([1, tqc], F32, tag="ps_z",
                                    name="ps_z", bufs=1)
                    nc.tensor.matmul(ps_z, lhsT=ones_col, rhs=acc,
                                     start=True, stop=True)
                    rz = psm.tile([1, tqc], F32, tag="rz", name="rz")
                    nc.vector.reciprocal_approx_fast(out=rz, in_=ps_z)
                    rzd = dram.tile([1, tqc], F32, tag="rzd", name="rzd",
                                    bufs=4)
                    nc.sync.dma_start(out=rzd, in_=rz)
                    rzb = pp.tile([HD, tqc], F32, tag="rzb", name="rzb",
                                  bufs=2)
                    nc.sync.dma_start(
                        out=rzb,
                        in_=bass.AP(
                            tensor=rzd.tensor,
                            offset=rzd.offset,
                            ap=[[0, HD]] + list(rzd.ap[1:]),
                        ),
                    )
                    nc.vector.tensor_mul(ot_tiles[h][:, qs], ps_o, rzb)

                pend_head = None  # (qc, h, acc, ps_o) awaiting Z/normalize
                for qc in range(ntc):
                    qs = slice(qc * tqc, (qc + 1) * tqc)
                    nkc = (qc + 1) * nmask
                    # new k columns for this q-chunk, all heads
                    for h in range(hpc):
                        nc.sync.dma_start(out=kt_res[h][:, qs],
                                          in_=kT_d[h, :, qs])
                    qt_tiles = []
                    for h in range(hpc):
                        qt_c = pqt.tile([HD, tqc], BF16, tag="qt", name="qt")
                        nc.sync.dma_start(out=qt_c, in_=qT_d[h, :, qs])
                        qt_tiles.append(qt_c)
                    for h in range(hpc):
                        qt_c = qt_tiles[h]
                        ps_o = psB.tile([HD, tqc], F32, tag="ps_o",
                                        name="ps_o", bufs=2)
                        acc = pac.tile([PB, tqc], F32R, tag="acc", name="acc")
                        pend_o = None  # (kc, pt) awaiting its O-matmul
                        for kc in range(nkc):
                            ps_s = psB.tile([PB, tqc], F32, tag="ps_s",
                                            name="ps_s", bufs=3)
                            nc.tensor.matmul(
                                ps_s,
                                lhsT=kt_res[h][:, kc * PB:(kc + 1) * PB],
                                rhs=qt_c,
                                start=True,
                                stop=True,
                            )
                            pt = pp.tile([PB, tqc], BF16, tag="pt", name="pt")
                            nc.scalar.activation(
                                out=pt, in_=ps_s, func=AF.Exp, scale=SCALE,
                            )
                            r = kc - qc * nmask
                            if r >= 0:
                                nc.vector.tensor_mul(pt, pt, masks[r])
                            if kc == 0:
                                nc.vector.tensor_copy(acc, pt)
                            else:
                                nc.vector.tensor_add(acc, acc, pt)
                            if kc == 1 and pend_head is not None:
                                finish_head(*pend_head)
                                pend_head = None
                            if pend_o is not None:
                                nc.tensor.matmul(
                                    ps_o,
                                    lhsT=v_all[:, pend_o[0], h, :],
                                    rhs=pend_o[1],
                                    start=(pend_o[0] == 0),
                                    stop=False,
                                )
                            pend_o = (kc, pt)
                        nc.tensor.matmul(
                            ps_o,
                            lhsT=v_all[:, pend_o[0], h, :],
                            rhs=pend_o[1],
                            start=(pend_o[0] == 0),
                            stop=True,
                        )
                        pend_head = (qc, h, acc, ps_o)
                        # interleave two proj groups of the previous q-chunk
                        if qc > 0:
                            proj_group(qc - 1, 2 * h)
                            proj_group(qc - 1, 2 * h + 1)
                finish_head(*pend_head)
                # tail: proj for the last q-chunk
                for g in range(2 * hpc):
                    proj_group(ntc - 1, g)

    if compile:
        nc.compile()
    return nc


def make_in_maps(x, cos, sin, W_qkv, b_qkv, W_proj):
    """Host-side sharding: pack the 8 per-core input dicts into the exact
    SBUF layouts the kernel consumes (bf16, partition-major)."""
    d = x.shape[-1]
    dck = d // PB
    ntc = T // TQC
    nvc = HPC * HD // TQC
    npq = d // TQC
    in_maps = []
    cosT = np.ascontiguousarray(cos.reshape(-1, HD).T).astype(np.float32)
    sinT = np.ascontiguousarray(sin.reshape(-1, HD).T).astype(np.float32)
    sinTs = sinT.copy()
    sinTs[: HD // 2] = -sinTs[: HD // 2]
    cosT = cosT.astype(NPBF)
    sinTs = sinTs.astype(NPBF)
    Wq = np.asarray(W_qkv[:, 0 * d:1 * d], np.float32)
    Wk = np.asarray(W_qkv[:, 1 * d:2 * d], np.float32)
    Wv = np.asarray(W_qkv[:, 2 * d:3 * d], np.float32)
    bqf = np.asarray(b_qkv[0 * d:1 * d], np.float32)
    bkf = np.asarray(b_qkv[1 * d:2 * d], np.float32)

    def pack_w_qk(w):                      # [d, hw] -> [PB, hpc, dck, HD]
        return np.ascontiguousarray(
            w.reshape(dck, PB, HPC, HD).transpose(1, 2, 0, 3)
        ).astype(NPBF)

    def pack_w_v(w):                       # [d, hw] -> [PB, nvc, dck, TQC]
        return np.ascontiguousarray(
            w.reshape(dck, PB, nvc, TQC).transpose(1, 2, 0, 3)
        ).astype(NPBF)

    def pack_wp(w):                        # [hw, d] -> [PB, npq, hpc, TQC]
        return np.ascontiguousarray(
            w.reshape(HPC, PB, npq, TQC).transpose(1, 2, 0, 3)
        ).astype(NPBF)

    def pack_x(xb):                        # [T, d] -> [PB, ntc, dck, TQC]
        return np.ascontiguousarray(
            xb.T.reshape(dck, PB, ntc, TQC).transpose(1, 2, 0, 3)
        ).astype(NPBF)

    for c in range(N_CORES):
        b = c // 2
        g = c % 2
        hw = HPC * HD
        cs = slice(g * hw, (g + 1) * hw)
        in_maps.append(
            {
                "xp": pack_x(np.asarray(x[b], np.float32)),
                "wq": pack_w_qk(Wq[:, cs]),
                "wk": pack_w_qk(Wk[:, cs]),
                "wv": pack_w_v(Wv[:, cs]),
                "wp": pack_wp(np.asarray(W_proj, np.float32)[cs, :]),
                "bq": np.ascontiguousarray(
                    bqf[cs].reshape(HPC, HD).T, dtype=np.float32),
                "bk": np.ascontiguousarray(
                    bkf[cs].reshape(HPC, HD).T, dtype=np.float32),
                "cosT": cosT,
                "sinTs": sinTs,
            }
        )
    return in_maps


def gather_output(results, b_qkv, W_proj, b_proj):
    """Sum the per-core partials and add the bias terms."""
    d = W_proj.shape[1]
    # v-bias contributes (sum_k attn = 1) exactly b_v @ W_proj per token.
    host_bias = (
        np.asarray(b_qkv[2 * d: 3 * d], np.float32) @ np.asarray(W_proj, np.float32)
        + np.asarray(b_proj, np.float32)
    )
    y = np.empty((B, T, d), np.float32)
    for b in range(B):
        y[b] = results[2 * b]["y"] + results[2 * b + 1]["y"] + host_bias
    return y


_NC_CACHE = {}


def kernel(x, cos, sin, W_qkv, b_qkv, W_proj, b_proj):
    x = np.asarray(x, np.float32)
    key = "full"
    if key not in _NC_CACHE:
        _NC_CACHE[key] = build_nc()
    nc = _NC_CACHE[key]
    in_maps = make_in_maps(
        x,
        np.asarray(cos, np.float32),
        np.asarray(sin, np.float32),
        np.asarray(W_qkv, np.float32),
        np.asarray(b_qkv, np.float32),
        np.asarray(W_proj, np.float32),
    )
    res = run_bass_kernel_spmd(nc, in_maps, core_ids=list(range(N_CORES)))
    return gather_output(res.results, b_qkv, W_proj, b_proj)


if __name__ == "__main__":
    import reference

    inputs = reference.setup_inputs()
    out = kernel(**{k: np.asarray(v) for k, v in inputs.items()})
    exp = np.asarray(reference.reference(**inputs))
    err = np.abs(out - exp).max() / np.abs(exp).max()
    print("rel err:", err)


# revision 7
# speedup vs baseline: 1.0913x; 1.0913x over previous
"""Causal self-attention with RoPE on 8 Trainium2 NeuronCores.

Sharding: tensor-parallel over heads x data-parallel over batch.
  core c -> batch b = c // 2, head-group g = c % 2 (heads 8g .. 8g+7).
Each core computes qkv projections for its 8 heads, RoPE, causal
attention, and a *partial* output projection (its heads' contribution
to y[b]). Host sums the two partials per batch and adds the bias
terms (b_proj and the v-bias routed through W_proj).

v2 design (vs the f32r baseline):
  * All PE operands are bf16 (1 cycle/row, same as f32r, but half the
    DMA/SBUF footprint). PSUM accumulation stays fp32.
  * Host prepacks x/weights into the exact SBUF tile layouts so every
    DMA moves large contiguous per-partition segments.
  * Phase A is a single fused pass over x^T chunks: q, k and v
    projections share each x chunk (x read once, not three times).
    RoPE rotate-matmuls are software-pipelined one head behind the
    accumulation groups so the PE never waits on the ACT eviction.
  * Attention runs q-chunk-outer / head-inner.  Softmax denominators
    are built by accumulating the exp tiles on the Vector engine and
    reducing with ONE ones-matmul per (head, q-chunk) instead of one
    per k-block (saves ~70us of PE time).  Causal masking multiplies
    bf16 0/1 masks on the DVE fast path.
  * The output projection for q-chunk qc-1 is interleaved into the
    attention head loop of q-chunk qc, so the proj never waits and
    PE gaps from the exp dependency chain get filled.
"""

import numpy as np
import ml_dtypes

import concourse.bass as bass
import concourse.mybir as mybir
import concourse.tile as tile
from concourse import bacc
from concourse.bass_utils import run_bass_kernel_spmd

F32 = mybir.dt.float32
F32R = mybir.dt.float32r
F16 = mybir.dt.float16
BF16 = mybir.dt.bfloat16
AF = mybir.ActivationFunctionType
ALU = mybir.AluOpType

D_MODEL = 2048
N_HEADS = 16
HD = 128
B, T = 4, 2048
N_CORES = 8
HPC = 8           # heads per core
TQC = 512         # query-chunk (free dim of S^T blocks)
PB = 128          # partitions / k-chunk
SCALE = 1.0 / np.sqrt(HD)
NPBF = np.dtype(ml_dtypes.bfloat16)


def build_nc(t=T, d=D_MODEL, hpc=HPC, tqc=TQC, compile=True):
    """Build the per-core Bass module. All 8 cores run this same module on
    different input slices."""
    nc = bacc.Bacc(trn_type="TRN2", target_bir_lowering=False)

    dck = d // PB          # D-chunks (contraction tiles)
    ntc = t // tqc         # t-chunks of tqc
    ntc128 = t // PB       # t-chunks of 128
    nmask = tqc // PB      # partial-block masks per q-chunk
    nvc = hpc * HD // tqc  # v output column chunks (2)
    hps = tqc // HD        # heads per v column chunk (4)
    npq = d // tqc         # proj output column chunks of tqc

    # Host-prepacked inputs (bf16, partition-major, contiguous segments).
    xp = nc.dram_tensor("xp", [PB, ntc, dck, tqc], BF16, kind="ExternalInput")
    wq = nc.dram_tensor("wq", [PB, hpc, dck, HD], BF16, kind="ExternalInput")
    wk = nc.dram_tensor("wk", [PB, hpc, dck, HD], BF16, kind="ExternalInput")
    wv = nc.dram_tensor("wv", [PB, nvc, dck, tqc], BF16, kind="ExternalInput")
    wp = nc.dram_tensor("wp", [PB, npq, hpc, tqc], BF16, kind="ExternalInput")
    bq = nc.dram_tensor("bq", [HD, hpc], F32, kind="ExternalInput")
    bk = nc.dram_tensor("bk", [HD, hpc], F32, kind="ExternalInput")
    cosT = nc.dram_tensor("cosT", [HD, t], BF16, kind="ExternalInput")
    # sinTs is sign-folded on host: rows 0:64 negated.
    sinTs = nc.dram_tensor("sinTs", [HD, t], BF16, kind="ExternalInput")
    y = nc.dram_tensor("y", [t, d], BF16, kind="ExternalOutput")

    with tile.TileContext(nc) as tc:
        with (
            tc.tile_pool(name="dram", bufs=1, space="DRAM") as dram,
            tc.tile_pool(name="consts", bufs=1) as consts,
            tc.tile_pool(name="vall", bufs=1) as pva,
        ):
            qT_d = dram.tile([hpc, HD, t], BF16, tag="qT_d")
            kT_d = dram.tile([hpc, HD, t], BF16, tag="kT_d")
            # v stays resident in SBUF from phase A through attention.
            v_all = pva.tile([PB, ntc128, hpc, HD], BF16, tag="v_all")

            # --- constants -------------------------------------------------
            ones_f = consts.tile([PB, 1], F32, tag="ones_f")
            nc.vector.memset(ones_f, 1.0)
            ones_col = consts.tile([PB, 1], F16, tag="ones")
            nc.vector.tensor_copy(ones_col, ones_f)
            # rotate_half as a matmul constant: rot^T = RT.T @ qT with
            # RT a pure 64-rotation permutation (signs live in sinTs,
            # whose first 64 rows are negated on the host).
            rT_f = consts.tile([HD, HD], F32, tag="rT_f")
            nc.gpsimd.memset(rT_f, 0.0)
            nc.gpsimd.affine_select(
                out=rT_f, in_=rT_f, compare_op=ALU.not_equal, fill=1.0,
                base=64, pattern=[[1, HD]], channel_multiplier=-1,
            )
            nc.gpsimd.affine_select(
                out=rT_f, in_=rT_f, compare_op=ALU.not_equal, fill=1.0,
                base=-64, pattern=[[1, HD]], channel_multiplier=-1,
            )
            rT = consts.tile([HD, HD], BF16, tag="rT")
            nc.vector.tensor_copy(rT, rT_f)
            # causal masks for the nmask partial diagonal blocks:
            # mask_r[p, f] = 1.0 if f >= p + r*128 else 0.0  (bf16)
            masks = []
            mk_f = consts.tile([PB, tqc], F32, tag="mk_f")
            for r in range(nmask):
                nc.gpsimd.memset(mk_f, 1.0)
                nc.gpsimd.affine_select(
                    out=mk_f, in_=mk_f,
                    compare_op=ALU.is_ge,
                    fill=0.0,
                    base=-(r * PB),
                    pattern=[[1, tqc]],
                    channel_multiplier=-1,
                )
                mk = consts.tile([PB, tqc], BF16, tag=f"mask{r}", name=f"mask{r}")
                nc.vector.tensor_copy(mk, mk_f)
                masks.append(mk)

            cosT_s = consts.tile([HD, t], BF16, tag="cosT")
            sinT_s = consts.tile([HD, t], BF16, tag="sinT")
            bq_s = consts.tile([HD, hpc], F32, tag="bq")
            bk_s = consts.tile([HD, hpc], F32, tag="bk")

            # ================= Phase A: fused QKV projections =============
            with (
                tc.tile_pool(name="w_q", bufs=1) as pwq,
                tc.tile_pool(name="w_k", bufs=1) as pwk,
                tc.tile_pool(name="w_v", bufs=1) as pwv,
                tc.tile_pool(name="x_a", bufs=2) as px,
                tc.tile_pool(name="raw_a", bufs=3) as pra,
                tc.tile_pool(name="rope_a", bufs=3) as pro,
                tc.tile_pool(name="psA", bufs=1, space="PSUM") as psA,
            ):
                wq_s = pwq.tile([PB, hpc, dck, HD], BF16, tag="wq_s")
                wk_s = pwk.tile([PB, hpc, dck, HD], BF16, tag="wk_s")
                wv_s = pwv.tile([PB, nvc, dck, tqc], BF16, tag="wv_s")

                # First-consumed-first DMA order: interleave x chunk 0 and
                # wq head 0 at fine grain so the first accumulation group
                # starts within ~2us, then remaining wq heads, wk, wv.
                xt_tiles = [None] * ntc
                xt_tiles[0] = px.tile([PB, dck, tqc], BF16, tag="xt", name="xt")
                for cc in range(0, dck, 4):
                    nc.sync.dma_start(
                        out=xt_tiles[0][:, cc:cc + 4, :],
                        in_=xp.ap()[:, 0, cc:cc + 4, :],
                    )
                    nc.sync.dma_start(
                        out=wq_s[:, 0, cc:cc + 4, :],
                        in_=wq.ap()[:, 0, cc:cc + 4, :],
                    )
                for h in range(1, hpc):
                    nc.sync.dma_start(out=wq_s[:, h], in_=wq.ap()[:, h])
                nc.sync.dma_start(out=cosT_s, in_=cosT.ap())
                nc.sync.dma_start(out=sinT_s, in_=sinTs.ap())
                nc.sync.dma_start(out=bq_s, in_=bq.ap())
                nc.sync.dma_start(out=bk_s, in_=bk.ap())
                for h in range(hpc):
                    nc.sync.dma_start(out=wk_s[:, h], in_=wk.ap()[:, h])
                for v in range(nvc):
                    nc.sync.dma_start(out=wv_s[:, v], in_=wv.ap()[:, v])

                def rope_finish(kind, h, tci, raw):
                    """rot-matmul + rope combine + DMA out for one head-chunk."""
                    ts = slice(tci * tqc, (tci + 1) * tqc)
                    ps_r = psA.tile([HD, tqc], F32, tag="ps_r", name="ps_r",
                                    bufs=2)
                    nc.tensor.matmul(ps_r, lhsT=rT, rhs=raw, start=True,
                                     stop=True)
                    rsin = pro.tile([HD, tqc], BF16, tag="rsin", name="rsin")
                    nc.vector.tensor_mul(rsin, ps_r, sinT_s[:, ts])
                    cosq = pro.tile([HD, tqc], BF16, tag="cosq", name="cosq")
                    nc.vector.tensor_mul(cosq, raw, cosT_s[:, ts])
                    out_t = pro.tile([HD, tqc], BF16, tag="outT", name="out_t")
                    nc.vector.tensor_add(out_t, cosq, rsin)
                    dst = qT_d if kind == "q" else kT_d
                    nc.sync.dma_start(out=dst[h, :, ts], in_=out_t)

                for tci in range(ntc):
                    xt_s = xt_tiles[tci]
                    for kind, w_s, bias_s in (("q", wq_s, bq_s),
                                              ("k", wk_s, bk_s)):
                        pend = None
                        for h in range(hpc):
                            ps = psA.tile([PB, tqc], F32, tag="ps_a",
                                          name="ps_a", bufs=3)
                            for c in range(dck):
                                nc.tensor.matmul(
                                    ps,
                                    lhsT=w_s[:, h, c, :],
                                    rhs=xt_s[:, c, :],
                                    start=(c == 0),
                                    stop=(c == dck - 1),
                                )
                            raw = pra.tile([PB, tqc], BF16, tag="raw",
                                           name="raw")
                            nc.scalar.activation(
                                out=raw, in_=ps, func=AF.Identity,
                                bias=bias_s[:, h:h + 1], scale=1.0,
                            )
                            if pend is not None:
                                rope_finish(kind, h - 1, tci, pend)
                            pend = raw
                        rope_finish(kind, hpc - 1, tci, pend)
                        if kind == "q":
                            # prefetch next x chunk mid-iteration
                            if tci + 1 < ntc:
                                nxt = px.tile([PB, dck, tqc], BF16, tag="xt",
                                              name="xt")
                                for cc in range(0, dck, 4):
                                    nc.sync.dma_start(
                                        out=nxt[:, cc:cc + 4, :],
                                        in_=xp.ap()[:, tci + 1, cc:cc + 4, :],
                                    )
                                xt_tiles[tci + 1] = nxt
                    # v projection for this x chunk
                    for t128 in range(tqc // PB):
                        kcg = tci * (tqc // PB) + t128
                        for nci in range(nvc):
                            ps_v = psA.tile([PB, tqc], F32, tag="ps_v",
                                            name="ps_v", bufs=3)
                            for c in range(dck):
                                nc.tensor.matmul(
                                    ps_v,
                                    lhsT=xt_s[:, c, t128 * PB:(t128 + 1) * PB],
                                    rhs=wv_s[:, nci, c, :],
                                    start=(c == 0),
                                    stop=(c == dck - 1),
                                )
                            nc.scalar.copy(
                                v_all[:, kcg, nci * hps:(nci + 1) * hps, :],
                                ps_v,
                            )

            # =========== Phase B+C: attention + interleaved proj ==========
            with (
                tc.tile_pool(name="ktq", bufs=1) as pkt,
                tc.tile_pool(name="qt_p", bufs=3) as pqt,
                tc.tile_pool(name="ot", bufs=1) as pot,
                tc.tile_pool(name="wp_p", bufs=1) as pwp,
                tc.tile_pool(name="pt_pool", bufs=4) as pp,
                tc.tile_pool(name="acc_p", bufs=2) as pac,
                tc.tile_pool(name="small", bufs=2) as psm,
                tc.tile_pool(name="yout", bufs=3) as py,
                tc.tile_pool(name="psB", bufs=1, space="PSUM") as psB,
            ):
                wp_s = pwp.tile([PB, npq, hpc, tqc], BF16, tag="wp_s")
                for nci in range(npq):
                    nc.sync.dma_start(out=wp_s[:, nci], in_=wp.ap()[:, nci])
                kt_res = [
                    pkt.tile([HD, t], BF16, tag=f"kt{h}", name=f"kt{h}")
                    for h in range(hpc)
                ]
                ot_tiles = [
                    pot.tile([HD, t], BF16, tag=f"ot{h}", name=f"ot{h}")
                    for h in range(hpc)
                ]

                def proj_group(qproj, g):
                    """One output-projection PSUM group for q-range qproj."""
                    t128 = 4 * qproj + g // npq
                    nci = g % npq
                    ps_y = psB.tile([PB, tqc], F32, tag="ps_y", name="ps_y",
                                    bufs=2)
                    for i in range(hpc):
                        hh = i  # freshest head (h7) last
                        nc.tensor.matmul(
                            ps_y,
                            lhsT=ot_tiles[hh][:, t128 * PB:(t128 + 1) * PB],
                            rhs=wp_s[:, nci, hh, :],
                            start=(i == 0),
                            stop=(i == hpc - 1),
                        )
                    y_t = py.tile([PB, tqc], BF16, tag="y_t", name="y_t")
                    nc.vector.tensor_copy(y_t, ps_y)
                    nc.sync.dma_start(
                        out=y.ap()[t128 * PB:(t128 + 1) * PB,
                                   nci * tqc:(nci + 1) * tqc],
                        in_=y_t,
                    )

                def finish_head(qc, h, acc, ps_o):
                    """Z-matmul + normalize for a finished head (emitted one
                    head late so the PE never waits on the exp/acc chain)."""
                    qs = slice(qc * tqc, (qc + 1) * tqc)
                    ps_z = psB.tile([1, tqc], F32, tag="ps_z",
                                    name="ps_z", bufs=1)
                    nc.tensor.matmul(ps_z, lhsT=ones_col, rhs=acc,
                                     start=True, stop=True)
                    rz = psm.tile([1, tqc], F32, tag="rz", name="rz")
                    nc.vector.reciprocal_approx_fast(out=rz, in_=ps_z)
                    rzd = dram.tile([1, tqc], F32, tag="rzd", name="rzd",
                                    bufs=4)
                    nc.sync.dma_start(out=rzd, in_=rz)
                    rzb = pp.tile([HD, tqc], F32, tag="rzb", name="rzb",
                                  bufs=2)
                    nc.sync.dma_start(
                        out=rzb,
                        in_=bass.AP(
                            tensor=rzd.tensor,
                            offset=rzd.offset,
                            ap=[[0, HD]] + list(rzd.ap[1:]),
                        ),
                    )
                    nc.vector.tensor_mul(ot_tiles[h][:, qs], ps_o, rzb)

                pend_head = None  # (qc, h, acc, ps_o) awaiting Z/normalize
                for qc in range(ntc):
                    qs = slice(qc * tqc, (qc + 1) * tqc)
                    nkc = (qc + 1) * nmask
                    # new k columns for this q-chunk, all heads
                    for h in range(hpc):
                        nc.sync.dma_start(out=kt_res[h][:, qs],
                                          in_=kT_d[h, :, qs])
                    qt_tiles = []
                    for h in range(hpc):
                        qt_c = pqt.tile([HD, tqc], BF16, tag="qt", name="qt")
                        nc.sync.dma_start(out=qt_c, in_=qT_d[h, :, qs])
                        qt_tiles.append(qt_c)
                    for h in range(hpc):
                        qt_c = qt_tiles[h]
                        ps_o = psB.tile([HD, tqc], F32, tag="ps_o",
                                        name="ps_o", bufs=2)
                        acc = pac.tile([PB, tqc], F16, tag="acc", name="acc")
                        pend_o = None  # (kc, pt) awaiting its O-matmul
                        for kc in range(nkc):
                            ps_s = psB.tile([PB, tqc], F32, tag="ps_s",
                                            name="ps_s", bufs=3)
                            nc.tensor.matmul(
                                ps_s,
                                lhsT=kt_res[h][:, kc * PB:(kc + 1) * PB],
                                rhs=qt_c,
                                start=True,
                                stop=True,
                            )
                            pt = pp.tile([PB, tqc], BF16, tag="pt", name="pt")
                            nc.scalar.activation(
                                out=pt, in_=ps_s, func=AF.Exp, scale=SCALE,
                            )
                            r = kc - qc * nmask
                            if r >= 0:
                                nc.vector.tensor_mul(pt, pt, masks[r])
                            if kc == 0:
                                nc.vector.tensor_copy(acc, pt)
                            else:
                                nc.vector.tensor_add(acc, acc, pt)
                            if kc == 1 and pend_head is not None:
                                finish_head(*pend_head)
                                pend_head = None
                            if pend_o is not None:
                                nc.tensor.matmul(
                                    ps_o,
                                    lhsT=v_all[:, pend_o[0], h, :],
                                    rhs=pend_o[1],
                                    start=(pend_o[0] == 0),
                                    stop=False,
                                )
                            pend_o = (kc, pt)
                        nc.tensor.matmul(
                            ps_o,
                            lhsT=v_all[:, pend_o[0], h, :],
                            rhs=pend_o[1],
                            start=(pend_o[0] == 0),
                            stop=True,
                        )
                        pend_head = (qc, h, acc, ps_o)
                        # interleave two proj groups of the previous q-chunk
                        if qc > 0:
                            proj_group(qc - 1, 2 * h)
                            proj_group(qc - 1, 2 * h + 1)
                finish_head(*pend_head)
                # tail: proj for the last q-chunk
                for g in range(2 * hpc):
                    proj_group(ntc - 1, g)

    if compile:
        nc.compile()
    return nc


def make_in_maps(x, cos, sin, W_qkv, b_qkv, W_proj):
    """Host-side sharding: pack the 8 per-core input dicts into the exact
    SBUF layouts the kernel consumes (bf16, partition-major)."""
    d = x.shape[-1]
    dck = d // PB
    ntc = T // TQC
    nvc = HPC * HD // TQC
    npq = d // TQC
    in_maps = []
    cosT = np.ascontiguousarray(cos.reshape(-1, HD).T).astype(np.float32)
    sinT = np.ascontiguousarray(sin.reshape(-1, HD).T).astype(np.float32)
    sinTs = sinT.copy()
    sinTs[: HD // 2] = -sinTs[: HD // 2]
    cosT = cosT.astype(NPBF)
    sinTs = sinTs.astype(NPBF)
    Wq = np.asarray(W_qkv[:, 0 * d:1 * d], np.float32)
    Wk = np.asarray(W_qkv[:, 1 * d:2 * d], np.float32)
    Wv = np.asarray(W_qkv[:, 2 * d:3 * d], np.float32)
    bqf = np.asarray(b_qkv[0 * d:1 * d], np.float32)
    bkf = np.asarray(b_qkv[1 * d:2 * d], np.float32)

    def pack_w_qk(w):                      # [d, hw] -> [PB, hpc, dck, HD]
        return np.ascontiguousarray(
            w.reshape(dck, PB, HPC, HD).transpose(1, 2, 0, 3)
        ).astype(NPBF)

    def pack_w_v(w):                       # [d, hw] -> [PB, nvc, dck, TQC]
        return np.ascontiguousarray(
            w.reshape(dck, PB, nvc, TQC).transpose(1, 2, 0, 3)
        ).astype(NPBF)

    def pack_wp(w):                        # [hw, d] -> [PB, npq, hpc, TQC]
        return np.ascontiguousarray(
            w.reshape(HPC, PB, npq, TQC).transpose(1, 2, 0, 3)
        ).astype(NPBF)

    def pack_x(xb):                        # [T, d] -> [PB, ntc, dck, TQC]
        return np.ascontiguousarray(
            xb.T.reshape(dck, PB, ntc, TQC).transpose(1, 2, 0, 3)
        ).astype(NPBF)

    for c in range(N_CORES):
        b = c // 2
        g = c % 2
        hw = HPC * HD
        cs = slice(g * hw, (g + 1) * hw)
        in_maps.append(
            {
                "xp": pack_x(np.asarray(x[b], np.float32)),
                "wq": pack_w_qk(Wq[:, cs]),
                "wk": pack_w_qk(Wk[:, cs]),
                "wv": pack_w_v(Wv[:, cs]),
                "wp": pack_wp(np.asarray(W_proj, np.float32)[cs, :]),
                "bq": np.ascontiguousarray(
                    bqf[cs].reshape(HPC, HD).T, dtype=np.float32),
                "bk": np.ascontiguousarray(
                    bkf[cs].reshape(HPC, HD).T, dtype=np.float32),
                "cosT": cosT,
                "sinTs": sinTs,
            }
        )
    return in_maps


def gather_output(results, b_qkv, W_proj, b_proj):
    """Sum the per-core partials and add the bias terms."""
    d = W_proj.shape[1]
    # v-bias contributes (sum_k attn = 1) exactly b_v @ W_proj per token.
    host_bias = (
        np.asarray(b_qkv[2 * d: 3 * d], np.float32) @ np.asarray(W_proj, np.float32)
        + np.asarray(b_proj, np.float32)
    )
    y = np.empty((B, T, d), np.float32)
    for b in range(B):
        y[b] = (np.asarray(results[2 * b]["y"], np.float32)
                + np.asarray(results[2 * b + 1]["y"], np.float32)
                + host_bias)
    return y


_NC_CACHE = {}


def kernel(x, cos, sin, W_qkv, b_qkv, W_proj, b_proj):
    x = np.asarray(x, np.float32)
    key = "full"
    if key not in _NC_CACHE:
        _NC_CACHE[key] = build_nc()
    nc = _NC_CACHE[key]
    in_maps = make_in_maps(
        x,
        np.asarray(cos, np.float32),
        np.asarray(sin, np.float32),
        np.asarray(W_qkv, np.float32),
        np.asarray(b_qkv, np.float32),
        np.asarray(W_proj, np.float32),
    )
    res = run_bass_kernel_spmd(nc, in_maps, core_ids=list(range(N_CORES)))
    return gather_output(res.results, b_qkv, W_proj, b_proj)


if __name__ == "__main__":
    import reference

    inputs = reference.setup_inputs()
    out = kernel(**{k: np.asarray(v) for k, v in inputs.items()})
    exp = np.asarray(reference.reference(**inputs))
    err = np.abs(out - exp).max() / np.abs(exp).max()
    print("rel err:", err)
